# revision 29
# baseline (speedup 1.0000x reference)
"""Distance transform via per-radius box-sums (no serial wavefront).

D(p) = Chebyshev distance to nearest source = sum_{t=0}^{n-1} [boxsum_t(p)==0]
(boxsum_t = clamped (2t+1)x(2t+1) window sum; monotone in t). Per t the box
sum is separable: row-window from prefix-sum differences (host-precomputed
padded prefix P), column-window via a banded 0/1 matmul W_t. All t are
independent -> fully pipelined across engines.

Reconstruction (exact): first-touch iter t* = D-1, and the first-touch conv
value is s* = sum_{8-neighb q, clamped} K(q-p)·[D(q) < D(p)] (neighbors
differ by at most 1 in D; clamped taps give D(q)=D(p) -> contribute 0,
matching replicate padding). Vertical neighbor maps come from PE shift
matmuls. out = (s*>0) ? (D-1) - h*log(s*) : 0.
"""

import math

import numpy as np

H_PARAM = 0.35
_PROGRAM_CACHE = {}


def _needed_iters(flat):
    B = flat > 0
    n = 0
    while n < 128 and not B.all():
        P = np.pad(B, ((0, 0), (1, 1), (1, 1)), mode="edge")
        D = np.zeros_like(B)
        for dy in range(3):
            for dx in range(3):
                D |= P[:, dy : dy + 128, dx : dx + 128]
        B = D
        n += 1
    return n


def _make_wmats(n):
    i = np.arange(128)
    d = np.abs(i[:, None] - i[None, :])
    return np.stack([(d <= t) for t in range(n)]).astype(np.float16)


def _make_shifts():
    # lhsT forms: DU = ShU @ D with DU[y] = D[max(y-1,0)];
    # DD[y] = D[min(y+1,127)]
    m = np.arange(128)
    ShU_T = np.zeros((128, 128), dtype=np.float16)
    ShU_T[np.maximum(m - 1, 0), m] = 1
    ShD_T = np.zeros((128, 128), dtype=np.float16)
    ShD_T[np.minimum(m + 1, 127), m] = 1
    return np.stack([ShU_T, ShD_T])


def _build(n):
    import concourse.bacc as bacc
    import concourse.tile as tile
    from concourse import mybir
    from concourse.alu_op_type import AluOpType as alu

    f32 = mybir.dt.float32
    f16 = mybir.dt.float16

    w1 = math.exp(-1.0 / H_PARAM)
    w2 = math.exp(-math.sqrt(2.0) / H_PARAM)

    PL = n + 1
    WIM = PL + 128 + n
    FWP = 3 * WIM
    FW = 3 * 130  # padded D layout for shifted taps

    nc = bacc.Bacc(
        "TRN2",
        target_bir_lowering=False,
        debug=False,
        enable_asserts=False,
        num_devices=8,
    )
    pd = nc.dram_tensor("p", [128, FWP], f16, kind="ExternalInput")
    wd = nc.dram_tensor("w", [n, 128, 128], f16, kind="ExternalInput")
    shd = nc.dram_tensor("sh", [2, 128, 128], f16, kind="ExternalInput")
    outd = nc.dram_tensor("out", [3, 128, 128], f32, kind="ExternalOutput")

    with tile.TileContext(nc) as tc:
        with (
            tc.tile_pool(name="state", bufs=1) as st,
            tc.tile_pool(name="work", bufs=4) as wk,
            tc.tile_pool(name="psum", bufs=4, space="PSUM") as pp,
        ):
            P = st.tile([128, FWP], f16, name="P")
            Ws = st.tile([128, n * 128], f16, name="Ws")
            Sh = st.tile([128, 2 * 128], f16, name="Sh")
            D = st.tile([128, 384], f16, name="D")  # contiguous [y,(c,x)]
            Dp = st.tile([128, FW], f16, name="Dp")  # padded for taps
            DU = st.tile([128, FW], f16, name="DU")
            DD = st.tile([128, FW], f16, name="DD")
            nc.sync.dma_start(P[:], pd.ap())
            for t in range(n):  # split so mm_t waits only its slab
                nc.sync.dma_start(Ws[:, t * 128 : (t + 1) * 128], wd.ap()[t])
            nc.vector.memset(D[:], 0.0)
            # preload the Ln activation table during phase 1
            warm = wk.tile([128, 1], f32, tag="warm")
            nc.vector.memset(warm[:], 1.0)
            warm2 = wk.tile([128, 1], f32, tag="warm2")
            nc.scalar.activation(
                warm2[:], warm[:], mybir.ActivationFunctionType.Ln
            )

            Pv = P[:].rearrange("p (c w) -> p c w", c=3)
            Dv = D[:].rearrange("p (c w) -> p c w", c=3)
            Dpv = Dp[:].rearrange("p (c w) -> p c w", c=3)

            # ---- phase 1: D = sum_t [boxsum_t == 0] ----
            for t in range(n):
                RW = wk.tile([128, 384], f16, tag="RW", bufs=6)
                eng = nc.vector if (t % 2 == 0) else nc.gpsimd
                eng.tensor_tensor(
                    RW[:].rearrange("p (c w) -> p c w", c=3),
                    Pv[:, :, PL + t : PL + t + 128],
                    Pv[:, :, PL - t - 1 : PL - t - 1 + 128],
                    op=alu.subtract,
                )
                ps = pp.tile([128, 384], f32, tag="ps", bufs=6)
                nc.tensor.matmul(
                    ps[:], Ws[:, t * 128 : (t + 1) * 128], RW[:],
                    start=True, stop=True,
                )
                # Dneg += sign(boxsum_t); D = n - Dneg (folded downstream)
                b = wk.tile([128, 384], f16, tag="b", bufs=6)
                nc.scalar.sign(b[:], ps[:])
                nc.vector.tensor_tensor(D[:], D[:], b[:], op=alu.add)

            # ---- phase 2 ----
            # padded copy of D (active cols + horizontal replicate pads)
            nc.vector.tensor_copy(Dpv[:, :, 1:129], Dv)
            nc.scalar.activation(
                Dpv[:, :, 0:130:129],
                Dpv[:, :, 1:129:127],
                mybir.ActivationFunctionType.Copy,
            )
            # vertical neighbor maps via PE shifts (replicate at rows 0/127)
            nc.sync.dma_start(
                Sh[:].rearrange("k (t m) -> k t m", t=2),
                shd.ap().rearrange("t k m -> k t m"),
            )
            psU = pp.tile([128, FW], f32, tag="psU", bufs=1)
            nc.tensor.matmul(psU[:], Sh[:, 0:128], Dp[:], start=True, stop=True)
            nc.scalar.activation(
                DU[:], psU[:], mybir.ActivationFunctionType.Copy
            )
            psD = pp.tile([128, FW], f32, tag="psD", bufs=1)
            nc.tensor.matmul(
                psD[:], Sh[:, 128:256], Dp[:], start=True, stop=True
            )
            nc.scalar.activation(
                DD[:], psD[:], mybir.ActivationFunctionType.Copy
            )
            DUv = DU[:].rearrange("p (c w) -> p c w", c=3)
            DDv = DD[:].rearrange("p (c w) -> p c w", c=3)

            def cmp(tp, tag):
                m = wk.tile([128, 384], f16, tag=tag)
                nc.vector.tensor_tensor(
                    m[:].rearrange("p (c w) -> p c w", c=3), tp, Dv,
                    op=alu.is_gt,
                )
                return m

            # horizontal taps first (only need Dp), then vertical/diagonal
            mL = cmp(Dpv[:, :, 0:128], "mL")
            mR = cmp(Dpv[:, :, 2:130], "mR")
            a1 = wk.tile([128, 384], f16, tag="a1")
            nc.vector.tensor_tensor(a1[:], mL[:], mR[:], op=alu.add)
            mU = cmp(DUv[:, :, 1:129], "mU")
            mUL = cmp(DUv[:, :, 0:128], "mUL")
            mUR = cmp(DUv[:, :, 2:130], "mUR")
            a2 = wk.tile([128, 384], f16, tag="a2")
            nc.vector.tensor_tensor(a2[:], mUL[:], mUR[:], op=alu.add)
            mD = cmp(DDv[:, :, 1:129], "mD")
            mDL = cmp(DDv[:, :, 0:128], "mDL")
            mDR = cmp(DDv[:, :, 2:130], "mDR")
            a3 = wk.tile([128, 384], f16, tag="a3")
            nc.vector.tensor_tensor(a3[:], mDL[:], mDR[:], op=alu.add)
            C4 = wk.tile([128, 384], f16, tag="C4")
            nc.vector.tensor_tensor(C4[:], mU[:], mD[:], op=alu.add)
            nc.vector.tensor_tensor(C4[:], C4[:], a1[:], op=alu.add)
            C8 = wk.tile([128, 384], f16, tag="C8")
            nc.vector.tensor_tensor(C8[:], a2[:], a3[:], op=alu.add)

            # sstar' = s*/w2 = (w1/w2)*C4 + C8; ln(s*) = ln(sstar') + ln(w2)
            sstar = wk.tile([128, 384], f32, tag="ss")
            nc.vector.scalar_tensor_tensor(
                sstar[:], C4[:], w1 / w2, C8[:], op0=alu.mult, op1=alu.add
            )

            # out = (s*>0) ? (n - Dneg - 1) - h*(ln(sstar') + ln(w2)) : 0
            sc = wk.tile([128, 384], f32, tag="sc")
            nc.vector.tensor_scalar_max(sc[:], sstar[:], 1e-30)
            lnS = wk.tile([128, 384], f32, tag="lnS")
            nc.scalar.activation(lnS[:], sc[:], mybir.ActivationFunctionType.Ln)
            u = wk.tile([128, 384], f32, tag="u")
            nc.vector.scalar_tensor_tensor(
                u[:], lnS[:], -H_PARAM, D[:], op0=alu.mult, op1=alu.subtract
            )
            v = wk.tile([128, 384], f32, tag="v")
            nc.vector.tensor_scalar_add(
                v[:], u[:], float(n - 1) - H_PARAM * math.log(w2)
            )
            outv = wk.tile([128, 384], f32, tag="outv")
            nc.vector.scalar_tensor_tensor(
                outv[:], sstar[:], 0.0, v[:], op0=alu.is_gt, op1=alu.mult
            )
            nc.sync.dma_start(
                outd.ap().rearrange("c h w -> h c w"),
                outv[:].rearrange("p (c w) -> p c w", c=3),
            )

    nc.compile()
    return nc


def _get_program(n):
    if n not in _PROGRAM_CACHE:
        _PROGRAM_CACHE[n] = _build(n)
    return _PROGRAM_CACHE[n]


def _prep_inputs(image, n):
    PL = n + 1
    WIM = PL + 128 + n
    x = (image > 0).astype(np.float64)
    P = np.cumsum(x, axis=-1)
    Ppad = np.zeros((3, 128, WIM), dtype=np.float16)
    Ppad[:, :, PL : PL + 128] = P
    Ppad[:, :, PL + 128 :] = P[:, :, 127:128]
    return np.ascontiguousarray(
        Ppad.transpose(1, 0, 2).reshape(128, 3 * WIM)
    )


def kernel(image):
    from concourse.bass_utils import run_bass_kernel_spmd

    image = np.ascontiguousarray(np.asarray(image), dtype=np.float32)
    assert image.shape == (8, 3, 128, 128)
    n = _needed_iters(image.reshape(24, 128, 128))
    if n == 0:
        return np.zeros_like(image)
    nc = _get_program(n)
    W = _make_wmats(n)
    Shm = _make_shifts()
    in_maps = [
        {"p": _prep_inputs(image[c], n), "w": W, "sh": Shm} for c in range(8)
    ]
    res = run_bass_kernel_spmd(nc, in_maps, core_ids=list(range(8)))
    return np.stack([res.results[c]["out"] for c in range(8)]).astype(
        np.float32
    )


# revision 30
# speedup vs baseline: 1.0011x; 1.0011x over previous
"""Distance transform via per-radius box-sums (no serial wavefront).

D(p) = Chebyshev distance to nearest source = sum_{t=0}^{n-1} [boxsum_t(p)==0]
(boxsum_t = clamped (2t+1)x(2t+1) window sum; monotone in t). Per t the box
sum is separable: row-window from prefix-sum differences (host-precomputed
padded prefix P), column-window via a banded 0/1 matmul W_t. All t are
independent -> fully pipelined across engines.

Reconstruction (exact): first-touch iter t* = D-1, and the first-touch conv
value is s* = sum_{8-neighb q, clamped} K(q-p)·[D(q) < D(p)] (neighbors
differ by at most 1 in D; clamped taps give D(q)=D(p) -> contribute 0,
matching replicate padding). Vertical neighbor maps come from PE shift
matmuls. out = (s*>0) ? (D-1) - h*log(s*) : 0.
"""

import math

import numpy as np

H_PARAM = 0.35
_PROGRAM_CACHE = {}


def _needed_iters(flat):
    B = flat > 0
    n = 0
    while n < 128 and not B.all():
        P = np.pad(B, ((0, 0), (1, 1), (1, 1)), mode="edge")
        D = np.zeros_like(B)
        for dy in range(3):
            for dx in range(3):
                D |= P[:, dy : dy + 128, dx : dx + 128]
        B = D
        n += 1
    return n


def _make_wmats(n):
    i = np.arange(128)
    d = np.abs(i[:, None] - i[None, :])
    return np.stack([(d <= t) for t in range(n)]).astype(np.float16)


def _make_shifts():
    # lhsT forms: DU = ShU @ D with DU[y] = D[max(y-1,0)];
    # DD[y] = D[min(y+1,127)]
    m = np.arange(128)
    ShU_T = np.zeros((128, 128), dtype=np.float16)
    ShU_T[np.maximum(m - 1, 0), m] = 1
    ShD_T = np.zeros((128, 128), dtype=np.float16)
    ShD_T[np.minimum(m + 1, 127), m] = 1
    return np.stack([ShU_T, ShD_T])


def _build(n):
    import concourse.bacc as bacc
    import concourse.tile as tile
    from concourse import mybir
    from concourse.alu_op_type import AluOpType as alu

    f32 = mybir.dt.float32
    f16 = mybir.dt.float16

    w1 = math.exp(-1.0 / H_PARAM)
    w2 = math.exp(-math.sqrt(2.0) / H_PARAM)

    PL = n + 1
    WIM = PL + 128 + n
    FWP = 3 * WIM
    FW = 3 * 130  # padded D layout for shifted taps

    nc = bacc.Bacc(
        "TRN2",
        target_bir_lowering=False,
        debug=False,
        enable_asserts=False,
        num_devices=8,
    )
    pd = nc.dram_tensor("p", [128, FWP], f16, kind="ExternalInput")
    wd = nc.dram_tensor("w", [n, 128, 128], f16, kind="ExternalInput")
    shd = nc.dram_tensor("sh", [2, 128, 128], f16, kind="ExternalInput")
    outd = nc.dram_tensor("out", [3, 128, 128], f32, kind="ExternalOutput")

    with tile.TileContext(nc) as tc:
        with (
            tc.tile_pool(name="state", bufs=1) as st,
            tc.tile_pool(name="work", bufs=4) as wk,
            tc.tile_pool(name="psum", bufs=4, space="PSUM") as pp,
        ):
            P = st.tile([128, FWP], f16, name="P")
            Ws = st.tile([128, n * 128], f16, name="Ws")
            Sh = st.tile([128, 2 * 128], f16, name="Sh")
            D = st.tile([128, 384], f16, name="D")  # contiguous [y,(c,x)]
            Dp = st.tile([128, FW], f16, name="Dp")  # padded for taps
            DU = st.tile([128, FW], f16, name="DU")
            DD = st.tile([128, FW], f16, name="DD")
            nc.sync.dma_start(P[:], pd.ap())
            for t in range(n):  # split so mm_t waits only its slab
                nc.sync.dma_start(Ws[:, t * 128 : (t + 1) * 128], wd.ap()[t])
            nc.vector.memset(D[:], 0.0)
            # preload the Ln activation table during phase 1
            warm = wk.tile([128, 1], f32, tag="warm")
            nc.vector.memset(warm[:], 1.0)
            warm2 = wk.tile([128, 1], f32, tag="warm2")
            nc.scalar.activation(
                warm2[:], warm[:], mybir.ActivationFunctionType.Ln
            )

            Pv = P[:].rearrange("p (c w) -> p c w", c=3)
            Dv = D[:].rearrange("p (c w) -> p c w", c=3)
            Dpv = Dp[:].rearrange("p (c w) -> p c w", c=3)

            # ---- phase 1: Dneg = sum_t sign(boxsum_t) ----
            # pairs of t share a 2-bank PSUM tile so one ACT sign covers both
            for t0 in range(0, n, 2):
                pair = [t for t in (t0, t0 + 1) if t < n]
                ps2 = pp.tile([128, 1024], f32, tag="ps", bufs=3)
                for j, t in enumerate(pair):
                    RW = wk.tile([128, 384], f16, tag="RW", bufs=6)
                    eng = nc.vector if (t % 2 == 0) else nc.gpsimd
                    eng.tensor_tensor(
                        RW[:].rearrange("p (c w) -> p c w", c=3),
                        Pv[:, :, PL + t : PL + t + 128],
                        Pv[:, :, PL - t - 1 : PL - t - 1 + 128],
                        op=alu.subtract,
                    )
                    nc.tensor.matmul(
                        ps2[:, j * 512 : j * 512 + 384],
                        Ws[:, t * 128 : (t + 1) * 128],
                        RW[:],
                        start=True, stop=True,
                    )
                b = wk.tile([128, 384 * len(pair)], f16, tag="b", bufs=4)
                if len(pair) == 2:
                    nc.scalar.sign(
                        b[:].rearrange("p (a w) -> p a w", a=2),
                        ps2[:].rearrange("p (a w) -> p a w", a=2)[:, :, 0:384],
                    )
                    nc.vector.tensor_tensor(D[:], D[:], b[:, 0:384], op=alu.add)
                    nc.vector.tensor_tensor(
                        D[:], D[:], b[:, 384:768], op=alu.add
                    )
                else:
                    nc.scalar.sign(b[:], ps2[:, 0:384])
                    nc.vector.tensor_tensor(D[:], D[:], b[:], op=alu.add)

            # ---- phase 2 ----
            # padded copy of D (active cols + horizontal replicate pads)
            nc.vector.tensor_copy(Dpv[:, :, 1:129], Dv)
            nc.scalar.activation(
                Dpv[:, :, 0:130:129],
                Dpv[:, :, 1:129:127],
                mybir.ActivationFunctionType.Copy,
            )
            # vertical neighbor maps via PE shifts (replicate at rows 0/127)
            nc.sync.dma_start(
                Sh[:].rearrange("k (t m) -> k t m", t=2),
                shd.ap().rearrange("t k m -> k t m"),
            )
            psU = pp.tile([128, FW], f32, tag="psU", bufs=1)
            nc.tensor.matmul(psU[:], Sh[:, 0:128], Dp[:], start=True, stop=True)
            nc.scalar.activation(
                DU[:], psU[:], mybir.ActivationFunctionType.Copy
            )
            psD = pp.tile([128, FW], f32, tag="psD", bufs=1)
            nc.tensor.matmul(
                psD[:], Sh[:, 128:256], Dp[:], start=True, stop=True
            )
            nc.scalar.activation(
                DD[:], psD[:], mybir.ActivationFunctionType.Copy
            )
            DUv = DU[:].rearrange("p (c w) -> p c w", c=3)
            DDv = DD[:].rearrange("p (c w) -> p c w", c=3)

            def cmp(tp, tag):
                m = wk.tile([128, 384], f16, tag=tag)
                nc.vector.tensor_tensor(
                    m[:].rearrange("p (c w) -> p c w", c=3), tp, Dv,
                    op=alu.is_gt,
                )
                return m

            # horizontal taps first (only need Dp), then vertical/diagonal
            mL = cmp(Dpv[:, :, 0:128], "mL")
            mR = cmp(Dpv[:, :, 2:130], "mR")
            a1 = wk.tile([128, 384], f16, tag="a1")
            nc.vector.tensor_tensor(a1[:], mL[:], mR[:], op=alu.add)
            mU = cmp(DUv[:, :, 1:129], "mU")
            mUL = cmp(DUv[:, :, 0:128], "mUL")
            mUR = cmp(DUv[:, :, 2:130], "mUR")
            a2 = wk.tile([128, 384], f16, tag="a2")
            nc.vector.tensor_tensor(a2[:], mUL[:], mUR[:], op=alu.add)
            mD = cmp(DDv[:, :, 1:129], "mD")
            mDL = cmp(DDv[:, :, 0:128], "mDL")
            mDR = cmp(DDv[:, :, 2:130], "mDR")
            a3 = wk.tile([128, 384], f16, tag="a3")
            nc.vector.tensor_tensor(a3[:], mDL[:], mDR[:], op=alu.add)
            C4 = wk.tile([128, 384], f16, tag="C4")
            nc.vector.tensor_tensor(C4[:], mU[:], mD[:], op=alu.add)
            nc.vector.tensor_tensor(C4[:], C4[:], a1[:], op=alu.add)
            C8 = wk.tile([128, 384], f16, tag="C8")
            nc.vector.tensor_tensor(C8[:], a2[:], a3[:], op=alu.add)

            # sstar' = s*/w2 = (w1/w2)*C4 + C8; ln(s*) = ln(sstar') + ln(w2)
            sstar = wk.tile([128, 384], f32, tag="ss")
            nc.vector.scalar_tensor_tensor(
                sstar[:], C4[:], w1 / w2, C8[:], op0=alu.mult, op1=alu.add
            )

            # out = (s*>0) ? (n - Dneg - 1) - h*(ln(sstar') + ln(w2)) : 0
            sc = wk.tile([128, 384], f32, tag="sc")
            nc.vector.tensor_scalar_max(sc[:], sstar[:], 1e-30)
            lnS = wk.tile([128, 384], f32, tag="lnS")
            nc.scalar.activation(lnS[:], sc[:], mybir.ActivationFunctionType.Ln)
            u = wk.tile([128, 384], f32, tag="u")
            nc.vector.scalar_tensor_tensor(
                u[:], lnS[:], -H_PARAM, D[:], op0=alu.mult, op1=alu.subtract
            )
            v = wk.tile([128, 384], f32, tag="v")
            nc.vector.tensor_scalar_add(
                v[:], u[:], float(n - 1) - H_PARAM * math.log(w2)
            )
            outv = wk.tile([128, 384], f32, tag="outv")
            nc.vector.scalar_tensor_tensor(
                outv[:], sstar[:], 0.0, v[:], op0=alu.is_gt, op1=alu.mult
            )
            nc.sync.dma_start(
                outd.ap().rearrange("c h w -> h c w"),
                outv[:].rearrange("p (c w) -> p c w", c=3),
            )

    nc.compile()
    return nc


def _get_program(n):
    if n not in _PROGRAM_CACHE:
        _PROGRAM_CACHE[n] = _build(n)
    return _PROGRAM_CACHE[n]


def _prep_inputs(image, n):
    PL = n + 1
    WIM = PL + 128 + n
    x = (image > 0).astype(np.float64)
    P = np.cumsum(x, axis=-1)
    Ppad = np.zeros((3, 128, WIM), dtype=np.float16)
    Ppad[:, :, PL : PL + 128] = P
    Ppad[:, :, PL + 128 :] = P[:, :, 127:128]
    return np.ascontiguousarray(
        Ppad.transpose(1, 0, 2).reshape(128, 3 * WIM)
    )


def kernel(image):
    from concourse.bass_utils import run_bass_kernel_spmd

    image = np.ascontiguousarray(np.asarray(image), dtype=np.float32)
    assert image.shape == (8, 3, 128, 128)
    n = _needed_iters(image.reshape(24, 128, 128))
    if n == 0:
        return np.zeros_like(image)
    nc = _get_program(n)
    W = _make_wmats(n)
    Shm = _make_shifts()
    in_maps = [
        {"p": _prep_inputs(image[c], n), "w": W, "sh": Shm} for c in range(8)
    ]
    res = run_bass_kernel_spmd(nc, in_maps, core_ids=list(range(8)))
    return np.stack([res.results[c]["out"] for c in range(8)]).astype(
        np.float32
    )
